# revision 5
# baseline (speedup 1.0000x reference)
"""Trainium2 Bass kernel for nn_Actor (gnn_message_passing), 8 NeuronCores.

Reference computation per batch row b:
    base = [l_emb(100), obs_body(10)]
    objs[k] = [onehot(k), obj_feats_k(15)]           k = 0..2
    for the 6 ordered pairs (i,j):
        z1_ij = [base, objs_i, objs_j] @ phi_w1 + phi_b1   (146 -> 256)
        h2_ij = relu(relu(z1_ij) @ phi_w2 + phi_b2)        (256 -> 256)
    agg  = sum_ij h2_ij
    r    = relu(agg @ rho_w1 + rho_b1)
    mean = r @ mean_w + mean_b
    lstd = clip(r @ lstd_w + lstd_b, -20, 2)

Strategy: pure data parallel over 8 cores (batch 65536 -> 8192/core).
Feature-major layout on chip (features on partitions, batch on the free
dim) so activations chain through the TensorEngine without transposes;
the host pre-transposes the inputs (same bytes, different strides).

Layer-1 algebra: z1_ij = A + B_ij where A is the shared [l_emb, body]
contribution (computed once per batch tile, K=111 incl. a ones-row
carrying phi_b1) and B_ij is the pair-specific part (K=32: two 16-row
blocks [feats_k, ones] whose ones-rows carry the one-hot contributions
a_i = phi_w1[110+i], g_j = phi_w1[128+j]). The 6 pairs are walked in a
Gray order, accumulating in PSUM with +/- transition matmuls
(-B_old + B_new, K=32), so layer 1 costs 7 matmuls per 128-feature
chunk instead of 24.
"""

import sys

sys.path.insert(0, "/opt/trn_rl_repo")

import numpy as np
import ml_dtypes

import concourse.tile as tile
from concourse import bacc, mybir
from concourse.bass_utils import run_bass_kernel_spmd

N_CORES = 8
B = 65536
B_LOC = B // N_CORES  # 8192
NB = 512
NT = B_LOC // NB  # 16

BF16 = mybir.dt.bfloat16
F32 = mybir.dt.float32
AF = mybir.ActivationFunctionType
ALU = mybir.AluOpType

# Gray walk over the 6 ordered pairs. xb holds the three distinct 32-row
# windows (PE needs operand base partitions at multiples of 32):
#   rows  0:32  = [block0; block1]
#   rows 32:64  = [block1; block2]
#   rows 64:96  = [block2; block0]
# XB_OFF[p] = window offset streamed by L1 matmul p (P0 + 5 transitions).
PAIRS = [(0, 1), (0, 2), (1, 2), (1, 0), (2, 0), (2, 1)]
XB_OFF = [0, 32, 0, 64, 32, 0]

LOG_SIG_MIN, LOG_SIG_MAX = -20.0, 2.0

TRACE = False  # set by an external harness to capture a profile
LAST_RESULT = None  # BassKernelResults of the most recent run

_nc_cache = None


def _build():
    nc = bacc.Bacc(None, target_bir_lowering=False)

    xa = nc.declare_dram_parameter("xa", [111, B_LOC], BF16, isOutput=False)
    xb = nc.declare_dram_parameter("xb", [96, B_LOC], BF16, isOutput=False)
    wa = nc.declare_dram_parameter("wa", [111, 256], BF16, isOutput=False)
    wb = nc.declare_dram_parameter("wb", [96, 12, 128], BF16, isOutput=False)
    w2 = nc.declare_dram_parameter("w2", [2, 2, 128, 128], BF16, isOutput=False)
    wr = nc.declare_dram_parameter("wr", [2, 2, 128, 128], BF16, isOutput=False)
    wh = nc.declare_dram_parameter("wh", [2, 128, 8], BF16, isOutput=False)
    b2 = nc.declare_dram_parameter("b2", [128, 2], F32, isOutput=False)
    rb = nc.declare_dram_parameter("rb", [128, 2], F32, isOutput=False)
    hb = nc.declare_dram_parameter("hb", [8, 1], F32, isOutput=False)
    cl = nc.declare_dram_parameter("cl", [8, 2], F32, isOutput=False)
    out = nc.declare_dram_parameter("out", [8, B_LOC], F32, isOutput=True)

    with tile.TileContext(nc) as tc:
        with (
            tc.tile_pool(name="consts", bufs=1) as consts,
            tc.tile_pool(name="xin", bufs=3) as xin,
            tc.tile_pool(name="hbuf", bufs=1) as hbuf,
            tc.tile_pool(name="psum", bufs=1, space="PSUM") as psum,
        ):
            wa_s = consts.tile([111, 256], BF16)
            nc.sync.dma_start(out=wa_s, in_=wa[:])
            wb_s = consts.tile([96, 12, 128], BF16)
            nc.sync.dma_start(out=wb_s, in_=wb[:])
            w2_s = consts.tile([128, 4, 128], BF16)
            nc.sync.dma_start(out=w2_s, in_=w2[:].rearrange("a m k n -> k (a m) n"))
            wr_s = consts.tile([128, 4, 128], BF16)
            nc.sync.dma_start(out=wr_s, in_=wr[:].rearrange("a m k n -> k (a m) n"))
            wh_s = consts.tile([128, 2, 8], BF16)
            nc.sync.dma_start(out=wh_s, in_=wh[:].rearrange("a k n -> k a n"))
            b2_s = consts.tile([128, 2], F32)
            nc.sync.dma_start(out=b2_s, in_=b2[:])
            rb_s = consts.tile([128, 2], F32)
            nc.sync.dma_start(out=rb_s, in_=rb[:])
            hb_s = consts.tile([8, 1], F32)
            nc.sync.dma_start(out=hb_s, in_=hb[:])
            cl_s = consts.tile([8, 2], F32)
            nc.sync.dma_start(out=cl_s, in_=cl[:])

            for t in range(NT):
                cols = slice(t * NB, (t + 1) * NB)
                xa_t = xin.tile([111, NB], BF16, tag="xa")
                nc.sync.dma_start(out=xa_t, in_=xa[:, cols])
                xb_t = xin.tile([96, NB], BF16, tag="xb")
                nc.sync.dma_start(out=xb_t, in_=xb[:, cols])

                # ---- phi layer 1: Gray-walk PSUM accumulation ----
                h1 = {}
                for mc in range(2):
                    z1 = psum.tile([128, NB], F32, tag="z1", bufs=2)
                    nc.tensor.matmul(
                        z1, wa_s[:, mc * 128 : (mc + 1) * 128], xa_t,
                        start=True, stop=False,
                    )
                    for p in range(6):
                        o = XB_OFF[p]
                        nc.tensor.matmul(
                            z1, wb_s[o : o + 32, p * 2 + mc, :],
                            xb_t[o : o + 32, :],
                            start=False, stop=(p == 5),
                        )
                        h1t = hbuf.tile([128, NB], BF16, tag="h1", bufs=18)
                        if (p + mc) % 2 == 0:
                            nc.scalar.activation(h1t, z1, AF.Relu)
                        else:
                            nc.vector.tensor_scalar_max(h1t, z1, 0.0)
                        h1[(p, mc)] = h1t

                # ---- phi layer 2 ----
                h2 = {}
                for p in range(6):
                    for mc in range(2):
                        z2 = psum.tile([128, NB], F32, tag="z2", bufs=3)
                        for kc in range(2):
                            nc.tensor.matmul(
                                z2, w2_s[:, kc * 2 + mc, :], h1[(p, kc)],
                                start=(kc == 0), stop=(kc == 1),
                            )
                        h2t = hbuf.tile([128, NB], BF16, tag="h2", bufs=16)
                        if (p + mc) % 2 == 0:
                            nc.scalar.activation(
                                h2t, z2, AF.Relu, bias=b2_s[:, mc : mc + 1]
                            )
                        else:
                            nc.vector.tensor_scalar(
                                out=h2t, in0=z2,
                                scalar1=b2_s[:, mc : mc + 1], scalar2=0.0,
                                op0=ALU.add, op1=ALU.max,
                            )
                        h2[(p, mc)] = h2t

                # ---- sum over the 6 pairs (DVE tree) ----
                agg = {}
                for mc in range(2):
                    t01 = hbuf.tile([128, NB], BF16, tag="ps", bufs=8)
                    nc.vector.tensor_add(t01, h2[(0, mc)], h2[(1, mc)])
                    t23 = hbuf.tile([128, NB], BF16, tag="ps", bufs=8)
                    nc.vector.tensor_add(t23, h2[(2, mc)], h2[(3, mc)])
                    t45 = hbuf.tile([128, NB], BF16, tag="ps", bufs=8)
                    nc.vector.tensor_add(t45, h2[(4, mc)], h2[(5, mc)])
                    t03 = hbuf.tile([128, NB], BF16, tag="ps", bufs=8)
                    nc.vector.tensor_add(t03, t01, t23)
                    ag = hbuf.tile([128, NB], BF16, tag="agg", bufs=4)
                    nc.vector.tensor_add(ag, t03, t45)
                    agg[mc] = ag

                # ---- rho layer ----
                r = {}
                for mc in range(2):
                    rz = psum.tile([128, NB], F32, tag="rz", bufs=2)
                    for kc in range(2):
                        nc.tensor.matmul(
                            rz, wr_s[:, kc * 2 + mc, :], agg[kc],
                            start=(kc == 0), stop=(kc == 1),
                        )
                    rt = hbuf.tile([128, NB], BF16, tag="r", bufs=4)
                    nc.scalar.activation(rt, rz, AF.Relu, bias=rb_s[:, mc : mc + 1])
                    r[mc] = rt

                # ---- heads: [mean | lstd] in one K=256 -> 8 matmul ----
                hz = psum.tile([8, NB], F32, tag="hz", bufs=1)
                for kc in range(2):
                    nc.tensor.matmul(
                        hz, wh_s[:, kc, :], r[kc], start=(kc == 0), stop=(kc == 1)
                    )
                out_s = hbuf.tile([8, NB], F32, tag="os", bufs=3)
                nc.scalar.activation(out_s, hz, AF.Identity, bias=hb_s)
                nc.vector.tensor_scalar(
                    out=out_s, in0=out_s,
                    scalar1=cl_s[:, 0:1], scalar2=cl_s[:, 1:2],
                    op0=ALU.max, op1=ALU.min,
                )
                nc.sync.dma_start(out=out[:, cols], in_=out_s)

    nc.finalize()
    return nc


def _prep_inputs(obs, l_emb, phi_w1, phi_b1, phi_w2, phi_b2,
                 rho_w1, rho_b1, mean_w, mean_b, lstd_w, lstd_b):
    bf = ml_dtypes.bfloat16
    f32 = np.float32
    obs = np.asarray(obs, f32)
    l_emb = np.asarray(l_emb, f32)
    W1 = np.asarray(phi_w1, f32)
    b1 = np.asarray(phi_b1, f32)

    ones = np.ones((1, B), f32)
    xa_full = np.concatenate([l_emb.T, obs[:, :10].T, ones], axis=0).astype(bf)
    feats = obs[:, 10:].reshape(B, 3, 15)
    blocks = []
    for k in (0, 1, 1, 2, 2, 0):
        blocks.append(feats[:, k, :].T)
        blocks.append(ones)
    xb_full = np.concatenate(blocks, axis=0).astype(bf)

    wa_np = np.concatenate([W1[:110], b1[None, :]], axis=0).astype(bf)  # [111,256]

    a = W1[110:113]   # one-hot rows, i-side
    Wfi = W1[113:128]
    g = W1[128:131]   # one-hot rows, j-side
    Wfj = W1[131:146]

    def blockw(side, k):
        if side == "i":
            return np.concatenate([Wfi, a[k][None]], axis=0)  # [16, 256]
        return np.concatenate([Wfj, g[k][None]], axis=0)

    mms = [
        np.concatenate([blockw("i", 0), blockw("j", 1)], 0),    # P0 (0,1)
        np.concatenate([-blockw("j", 1), blockw("j", 2)], 0),   # -> (0,2)
        np.concatenate([-blockw("i", 0), blockw("i", 1)], 0),   # -> (1,2)
        np.concatenate([-blockw("j", 2), blockw("j", 0)], 0),   # -> (1,0)
        np.concatenate([-blockw("i", 1), blockw("i", 2)], 0),   # -> (2,0)
        np.concatenate([-blockw("j", 0), blockw("j", 1)], 0),   # -> (2,1)
    ]
    wb_np = np.zeros((96, 12, 128), np.float32)
    xb_off = [0, 32, 0, 64, 32, 0]
    for p, m in enumerate(mms):  # m: [32, 256]
        o = xb_off[p]
        for mc in range(2):
            wb_np[o : o + 32, p * 2 + mc, :] = m[:, mc * 128 : (mc + 1) * 128]
    wb_np = wb_np.astype(bf)

    def kxm(w):  # [256, 256] -> [kc, mc, 128, 128]
        w = np.asarray(w, f32)
        return np.ascontiguousarray(
            w.reshape(2, 128, 2, 128).transpose(0, 2, 1, 3)
        ).astype(bf)

    w2_np = kxm(phi_w2)
    wr_np = kxm(rho_w1)
    wh_np = np.ascontiguousarray(
        np.concatenate([np.asarray(mean_w, f32), np.asarray(lstd_w, f32)], axis=1)
        .reshape(2, 128, 8)
    ).astype(bf)

    b2_np = np.ascontiguousarray(np.asarray(phi_b2, f32).reshape(2, 128).T)
    rb_np = np.ascontiguousarray(np.asarray(rho_b1, f32).reshape(2, 128).T)
    hb_np = np.concatenate(
        [np.asarray(mean_b, f32), np.asarray(lstd_b, f32)]
    ).reshape(8, 1)
    big = np.float32(3.0e38)
    cl_np = np.array(
        [[-big, big]] * 4 + [[LOG_SIG_MIN, LOG_SIG_MAX]] * 4, np.float32
    )

    shared = {
        "wa": wa_np, "wb": wb_np, "w2": w2_np, "wr": wr_np, "wh": wh_np,
        "b2": b2_np, "rb": rb_np, "hb": hb_np, "cl": cl_np,
    }
    in_maps = []
    for c in range(N_CORES):
        rows = slice(c * B_LOC, (c + 1) * B_LOC)
        m = dict(shared)
        m["xa"] = np.ascontiguousarray(xa_full[:, rows])
        m["xb"] = np.ascontiguousarray(xb_full[:, rows])
        in_maps.append(m)
    return in_maps


def kernel(obs, l_emb, phi_w1, phi_b1, phi_w2, phi_b2,
           rho_w1, rho_b1, mean_w, mean_b, lstd_w, lstd_b):
    global _nc_cache, LAST_RESULT
    if _nc_cache is None:
        _nc_cache = _build()
    in_maps = _prep_inputs(obs, l_emb, phi_w1, phi_b1, phi_w2, phi_b2,
                           rho_w1, rho_b1, mean_w, mean_b, lstd_w, lstd_b)
    res = run_bass_kernel_spmd(
        _nc_cache, in_maps, core_ids=list(range(N_CORES)), trace=TRACE
    )
    LAST_RESULT = res
    outs = np.concatenate(
        [res.results[c]["out"].T for c in range(N_CORES)], axis=0
    )  # [B, 8]
    mean = np.ascontiguousarray(outs[:, :4], dtype=np.float32)
    log_std = np.ascontiguousarray(outs[:, 4:8], dtype=np.float32)
    return mean, log_std


# revision 6
# speedup vs baseline: 1.4835x; 1.4835x over previous
"""Trainium2 Bass kernel for nn_Actor (gnn_message_passing), 8 NeuronCores.

Reference computation per batch row b:
    base = [l_emb(100), obs_body(10)]
    objs[k] = [onehot(k), obj_feats_k(15)]           k = 0..2
    for the 6 ordered pairs (i,j):
        z1_ij = [base, objs_i, objs_j] @ phi_w1 + phi_b1   (146 -> 256)
        h2_ij = relu(relu(z1_ij) @ phi_w2 + phi_b2)        (256 -> 256)
    agg  = sum_ij h2_ij
    r    = relu(agg @ rho_w1 + rho_b1)
    mean = r @ mean_w + mean_b
    lstd = clip(r @ lstd_w + lstd_b, -20, 2)

Strategy: pure data parallel over 8 cores (batch 65536 -> 8192/core).
Feature-major layout on chip (features on partitions, batch on the free
dim) so activations chain through the TensorEngine without transposes;
the host pre-transposes the inputs (same bytes, different strides).

Layer-1 algebra: z1_ij = A + B_ij where A is the shared [l_emb, body]
contribution (computed once per batch tile, K=111 incl. a ones-row
carrying phi_b1) and B_ij is the pair-specific part (K=32: two 16-row
blocks [feats_k, ones] whose ones-rows carry the one-hot contributions
a_i = phi_w1[110+i], g_j = phi_w1[128+j]). The 6 pairs are walked in a
Gray order, accumulating in PSUM with +/- transition matmuls
(-B_old + B_new, K=32), so layer 1 costs 7 matmuls per 128-feature
chunk instead of 24.
"""

import sys

sys.path.insert(0, "/opt/trn_rl_repo")

import numpy as np
import ml_dtypes

import concourse.tile as tile
from concourse import bacc, mybir
from concourse.bass_utils import run_bass_kernel_spmd

N_CORES = 8
B = 65536
B_LOC = B // N_CORES  # 8192
NB = 512
NT = B_LOC // NB  # 16

BF16 = mybir.dt.bfloat16
F32 = mybir.dt.float32
AF = mybir.ActivationFunctionType
ALU = mybir.AluOpType

# Gray walk over the 6 ordered pairs. xb holds the three distinct 32-row
# windows (PE needs operand base partitions at multiples of 32):
#   rows  0:32  = [block0; block1]
#   rows 32:64  = [block1; block2]
#   rows 64:96  = [block2; block0]
# XB_OFF[p] = window offset streamed by L1 matmul p (P0 + 5 transitions).
PAIRS = [(0, 1), (0, 2), (1, 2), (1, 0), (2, 0), (2, 1)]
XB_OFF = [0, 32, 0, 64, 32, 0]

LOG_SIG_MIN, LOG_SIG_MAX = -20.0, 2.0

TRACE = False  # set by an external harness to capture a profile
LAST_RESULT = None  # BassKernelResults of the most recent run

_nc_cache = None


def _build():
    nc = bacc.Bacc(None, target_bir_lowering=False)

    xa = nc.declare_dram_parameter("xa", [111, B_LOC], BF16, isOutput=False)
    xb = nc.declare_dram_parameter("xb", [96, B_LOC], BF16, isOutput=False)
    wa = nc.declare_dram_parameter("wa", [111, 256], BF16, isOutput=False)
    wb = nc.declare_dram_parameter("wb", [96, 12, 128], BF16, isOutput=False)
    w2 = nc.declare_dram_parameter("w2", [2, 2, 128, 128], BF16, isOutput=False)
    wr = nc.declare_dram_parameter("wr", [2, 2, 128, 128], BF16, isOutput=False)
    wh = nc.declare_dram_parameter("wh", [2, 128, 8], BF16, isOutput=False)
    b2 = nc.declare_dram_parameter("b2", [128, 2], F32, isOutput=False)
    rb = nc.declare_dram_parameter("rb", [128, 2], F32, isOutput=False)
    hb = nc.declare_dram_parameter("hb", [8, 1], F32, isOutput=False)
    cl = nc.declare_dram_parameter("cl", [8, 2], F32, isOutput=False)
    out = nc.declare_dram_parameter("out", [8, B_LOC], F32, isOutput=True)

    with tile.TileContext(nc) as tc:
        with (
            tc.tile_pool(name="consts", bufs=1) as consts,
            tc.tile_pool(name="xin", bufs=3) as xin,
            tc.tile_pool(name="hbuf", bufs=1) as hbuf,
            tc.tile_pool(name="psum", bufs=1, space="PSUM") as psum,
        ):
            wa_s = consts.tile([111, 256], BF16)
            nc.sync.dma_start(out=wa_s, in_=wa[:])
            wb_s = consts.tile([96, 12, 128], BF16)
            nc.sync.dma_start(out=wb_s, in_=wb[:])
            w2_s = consts.tile([128, 4, 128], BF16)
            nc.sync.dma_start(out=w2_s, in_=w2[:].rearrange("a m k n -> k (a m) n"))
            wr_s = consts.tile([128, 4, 128], BF16)
            nc.sync.dma_start(out=wr_s, in_=wr[:].rearrange("a m k n -> k (a m) n"))
            wh_s = consts.tile([128, 2, 8], BF16)
            nc.sync.dma_start(out=wh_s, in_=wh[:].rearrange("a k n -> k a n"))
            b2_s = consts.tile([128, 2], F32)
            nc.sync.dma_start(out=b2_s, in_=b2[:])
            rb_s = consts.tile([128, 2], F32)
            nc.sync.dma_start(out=rb_s, in_=rb[:])
            hb_s = consts.tile([8, 1], F32)
            nc.sync.dma_start(out=hb_s, in_=hb[:])
            cl_s = consts.tile([8, 2], F32)
            nc.sync.dma_start(out=cl_s, in_=cl[:])

            for t in range(NT):
                cols = slice(t * NB, (t + 1) * NB)
                xa_t = xin.tile([111, NB], BF16, tag="xa")
                nc.sync.dma_start(out=xa_t, in_=xa[:, cols])
                xb_t = xin.tile([96, NB], BF16, tag="xb")
                nc.sync.dma_start(out=xb_t, in_=xb[:, cols])

                # Software-pipelined emission: the PE instruction stream is
                # static, so between the Gray-chain transition matmuls of
                # pair p we place the L2 matmuls of pair p-1 — the chain's
                # PSUM drains then overlap with useful PE work instead of
                # stalling the engine (which also re-throttles the HAM
                # clock gate).
                z1 = {}
                h1 = {}
                h2 = {}
                agg = {}

                def l1_step(p):
                    for mc in range(2):
                        if p == 0:
                            z1[mc] = psum.tile(
                                [128, NB], F32, tag="z1", bufs=3,
                                name=f"z1_{t}_{mc}",
                            )
                            nc.tensor.matmul(
                                z1[mc], wa_s[:, mc * 128 : (mc + 1) * 128],
                                xa_t, start=True, stop=False,
                            )
                        o = XB_OFF[p]
                        nc.tensor.matmul(
                            z1[mc], wb_s[o : o + 32, p * 2 + mc, :],
                            xb_t[o : o + 32, :],
                            start=False, stop=(p == 5),
                        )
                        h1t = hbuf.tile(
                            [128, NB], BF16, tag="h1", bufs=18,
                            name=f"h1_{t}_{p}_{mc}",
                        )
                        nc.scalar.activation(h1t, z1[mc], AF.Relu)
                        h1[(p, mc)] = h1t

                def l2_step(q):
                    for mc in range(2):
                        z2 = psum.tile(
                            [128, NB], F32, tag="z2", bufs=3,
                            name=f"z2_{t}_{q}_{mc}",
                        )
                        for kc in range(2):
                            nc.tensor.matmul(
                                z2, w2_s[:, kc * 2 + mc, :], h1[(q, kc)],
                                start=(kc == 0), stop=(kc == 1),
                            )
                        h2t = hbuf.tile(
                            [128, NB], BF16, tag="h2", bufs=16,
                            name=f"h2_{t}_{q}_{mc}",
                        )
                        nc.vector.tensor_scalar(
                            out=h2t, in0=z2,
                            scalar1=b2_s[:, mc : mc + 1], scalar2=0.0,
                            op0=ALU.add, op1=ALU.max,
                        )
                        h2[(q, mc)] = h2t

                def psum_step(q):
                    # partial pair sums as soon as both inputs exist
                    for mc in range(2):
                        if q == 1:
                            s = hbuf.tile([128, NB], BF16, tag="ps", bufs=8,
                                          name=f"t01_{t}_{mc}")
                            nc.vector.tensor_add(s, h2[(0, mc)], h2[(1, mc)])
                            agg[("t01", mc)] = s
                        elif q == 3:
                            s = hbuf.tile([128, NB], BF16, tag="ps", bufs=8,
                                          name=f"t23_{t}_{mc}")
                            nc.vector.tensor_add(s, h2[(2, mc)], h2[(3, mc)])
                            agg[("t23", mc)] = s
                        elif q == 5:
                            s = hbuf.tile([128, NB], BF16, tag="ps", bufs=8,
                                          name=f"t45_{t}_{mc}")
                            nc.vector.tensor_add(s, h2[(4, mc)], h2[(5, mc)])
                            u = hbuf.tile([128, NB], BF16, tag="ps", bufs=8,
                                          name=f"t03_{t}_{mc}")
                            nc.vector.tensor_add(
                                u, agg[("t01", mc)], agg[("t23", mc)]
                            )
                            ag = hbuf.tile([128, NB], BF16, tag="agg", bufs=4,
                                           name=f"agg_{t}_{mc}")
                            nc.vector.tensor_add(ag, u, s)
                            agg[mc] = ag

                l1_step(0)
                for p in range(1, 7):
                    if p <= 5:
                        l1_step(p)
                    l2_step(p - 1)
                    psum_step(p - 1)

                # ---- rho layer ----
                r = {}
                for mc in range(2):
                    rz = psum.tile([128, NB], F32, tag="rz", bufs=2,
                                   name=f"rz_{t}_{mc}")
                    for kc in range(2):
                        nc.tensor.matmul(
                            rz, wr_s[:, kc * 2 + mc, :], agg[kc],
                            start=(kc == 0), stop=(kc == 1),
                        )
                    rt = hbuf.tile([128, NB], BF16, tag="r", bufs=4,
                                   name=f"r_{t}_{mc}")
                    nc.scalar.activation(rt, rz, AF.Relu, bias=rb_s[:, mc : mc + 1])
                    r[mc] = rt

                # ---- heads: [mean | lstd] in one K=256 -> 8 matmul ----
                hz = psum.tile([8, NB], F32, tag="rz", bufs=2,
                               name=f"hz_{t}")
                for kc in range(2):
                    nc.tensor.matmul(
                        hz, wh_s[:, kc, :], r[kc], start=(kc == 0), stop=(kc == 1)
                    )
                out_s = hbuf.tile([8, NB], F32, tag="os", bufs=3, name=f"os_{t}")
                nc.scalar.activation(out_s, hz, AF.Identity, bias=hb_s)
                nc.vector.tensor_scalar(
                    out=out_s, in0=out_s,
                    scalar1=cl_s[:, 0:1], scalar2=cl_s[:, 1:2],
                    op0=ALU.max, op1=ALU.min,
                )
                nc.sync.dma_start(out=out[:, cols], in_=out_s)

    nc.finalize()
    return nc


def _prep_inputs(obs, l_emb, phi_w1, phi_b1, phi_w2, phi_b2,
                 rho_w1, rho_b1, mean_w, mean_b, lstd_w, lstd_b):
    bf = ml_dtypes.bfloat16
    f32 = np.float32
    obs = np.asarray(obs, f32)
    l_emb = np.asarray(l_emb, f32)
    W1 = np.asarray(phi_w1, f32)
    b1 = np.asarray(phi_b1, f32)

    ones = np.ones((1, B), f32)
    xa_full = np.concatenate([l_emb.T, obs[:, :10].T, ones], axis=0).astype(bf)
    feats = obs[:, 10:].reshape(B, 3, 15)
    blocks = []
    for k in (0, 1, 1, 2, 2, 0):
        blocks.append(feats[:, k, :].T)
        blocks.append(ones)
    xb_full = np.concatenate(blocks, axis=0).astype(bf)

    wa_np = np.concatenate([W1[:110], b1[None, :]], axis=0).astype(bf)  # [111,256]

    a = W1[110:113]   # one-hot rows, i-side
    Wfi = W1[113:128]
    g = W1[128:131]   # one-hot rows, j-side
    Wfj = W1[131:146]

    def blockw(side, k):
        if side == "i":
            return np.concatenate([Wfi, a[k][None]], axis=0)  # [16, 256]
        return np.concatenate([Wfj, g[k][None]], axis=0)

    mms = [
        np.concatenate([blockw("i", 0), blockw("j", 1)], 0),    # P0 (0,1)
        np.concatenate([-blockw("j", 1), blockw("j", 2)], 0),   # -> (0,2)
        np.concatenate([-blockw("i", 0), blockw("i", 1)], 0),   # -> (1,2)
        np.concatenate([-blockw("j", 2), blockw("j", 0)], 0),   # -> (1,0)
        np.concatenate([-blockw("i", 1), blockw("i", 2)], 0),   # -> (2,0)
        np.concatenate([-blockw("j", 0), blockw("j", 1)], 0),   # -> (2,1)
    ]
    wb_np = np.zeros((96, 12, 128), np.float32)
    xb_off = [0, 32, 0, 64, 32, 0]
    for p, m in enumerate(mms):  # m: [32, 256]
        o = xb_off[p]
        for mc in range(2):
            wb_np[o : o + 32, p * 2 + mc, :] = m[:, mc * 128 : (mc + 1) * 128]
    wb_np = wb_np.astype(bf)

    def kxm(w):  # [256, 256] -> [kc, mc, 128, 128]
        w = np.asarray(w, f32)
        return np.ascontiguousarray(
            w.reshape(2, 128, 2, 128).transpose(0, 2, 1, 3)
        ).astype(bf)

    w2_np = kxm(phi_w2)
    wr_np = kxm(rho_w1)
    wh_np = np.ascontiguousarray(
        np.concatenate([np.asarray(mean_w, f32), np.asarray(lstd_w, f32)], axis=1)
        .reshape(2, 128, 8)
    ).astype(bf)

    b2_np = np.ascontiguousarray(np.asarray(phi_b2, f32).reshape(2, 128).T)
    rb_np = np.ascontiguousarray(np.asarray(rho_b1, f32).reshape(2, 128).T)
    hb_np = np.concatenate(
        [np.asarray(mean_b, f32), np.asarray(lstd_b, f32)]
    ).reshape(8, 1)
    big = np.float32(3.0e38)
    cl_np = np.array(
        [[-big, big]] * 4 + [[LOG_SIG_MIN, LOG_SIG_MAX]] * 4, np.float32
    )

    shared = {
        "wa": wa_np, "wb": wb_np, "w2": w2_np, "wr": wr_np, "wh": wh_np,
        "b2": b2_np, "rb": rb_np, "hb": hb_np, "cl": cl_np,
    }
    in_maps = []
    for c in range(N_CORES):
        rows = slice(c * B_LOC, (c + 1) * B_LOC)
        m = dict(shared)
        m["xa"] = np.ascontiguousarray(xa_full[:, rows])
        m["xb"] = np.ascontiguousarray(xb_full[:, rows])
        in_maps.append(m)
    return in_maps


def kernel(obs, l_emb, phi_w1, phi_b1, phi_w2, phi_b2,
           rho_w1, rho_b1, mean_w, mean_b, lstd_w, lstd_b):
    global _nc_cache, LAST_RESULT
    if _nc_cache is None:
        _nc_cache = _build()
    in_maps = _prep_inputs(obs, l_emb, phi_w1, phi_b1, phi_w2, phi_b2,
                           rho_w1, rho_b1, mean_w, mean_b, lstd_w, lstd_b)
    res = run_bass_kernel_spmd(
        _nc_cache, in_maps, core_ids=list(range(N_CORES)), trace=TRACE
    )
    LAST_RESULT = res
    outs = np.concatenate(
        [res.results[c]["out"].T for c in range(N_CORES)], axis=0
    )  # [B, 8]
    mean = np.ascontiguousarray(outs[:, :4], dtype=np.float32)
    log_std = np.ascontiguousarray(outs[:, 4:8], dtype=np.float32)
    return mean, log_std
